# revision 7
# baseline (speedup 1.0000x reference)
"""GPT one-layer (S=2048, D=2048, H=8, V=50257, DF=8192) on 8 TRN2 NeuronCores.

Sharding (tensor-parallel, per the megatron pattern):
  - embedding + lm_head: vocab-sharded (V padded to 51200 = 8*6400); embedding
    lookup is a masked local gather + bf16 AllReduce (one-hot masked partials,
    so the CCE adds are exact); lm_head produces logits columns per core.
  - attention: 1 head per core (dh=256). Scores are computed transposed
    (scoresT[t,s]) so probs land with t on partitions; attn = probsT.T @ [v|1]
    gives the softmax denominator for free in column 256. Scores are tiny
    (|s|<0.01) so exp without max-subtraction is exact.
  - LN affine (ln_g/ln_b) is folded into W1/b1 on the host (no residual after
    the MLP in this module, so the fold is exact).
  - MLP: W1 column-, W2 row-parallel (DF 8192 -> 1024/core); MLP2 emits
    o TRANSPOSED ([D, S]) which is AllReduced in fp32 (4 chunks over S) and
    consumed directly as the lm_head's lhsT.

All compute matmuls are bf16 with fp32 PSUM accumulation; LN/softmax/gelu
statistics in fp32.
"""

from contextlib import ExitStack

import numpy as np
import ml_dtypes

import concourse.bass as bass
import concourse.tile as tile
from concourse import mybir
from concourse.bass import ts, ds
from concourse.bass_utils import run_bass_kernel_spmd

P = 128
S = 2048
D = 2048
H = 8
DH = 256
V = 50257
VP = 6400          # vocab rows per core (padded)
VPAD = VP * 8
DF = 8192
DFS = DF // 8      # 1024 mlp hidden per core
NC = 8
KO = D // P        # 16 K-subtiles over D
ST = S // P        # 16 s-tiles
SC = 4             # s-chunks of 512
SCW = S // SC      # 512
FT = DFS // P      # 8 f-tiles
KF = DFS // P      # 8 K-subtiles over DF shard
LN_EPS = 1e-5

f32 = mybir.dt.float32
bf16 = mybir.dt.bfloat16
i32 = mybir.dt.int32
AF = mybir.ActivationFunctionType
OP = mybir.AluOpType


def fix_wait_limits(nc):
    """Hoist excess semaphore waits into standalone EventSemaphore insts.

    Several TRN2 instruction encodings (DMA triggers, self-loading fp32/fp32r
    matmuls) only have a single sync-wait slot; Tile can emit >1 wait on an
    instruction (data dep + DMA-lane ordering), which walrus rejects with
    "Too many sync wait commands". Each engine executes its stream in order,
    so a preceding EventSemaphore wait on the same engine is equivalent.
    """
    n_fixed = 0
    for f in nc.m.functions:
        for bb in f.blocks:
            out = []
            for inst in bb.instructions:
                si = getattr(inst, "sync_info", None)
                waits = list(si.on_wait) if si is not None and si.on_wait else []
                if len(waits) > 1:
                    for j, w in enumerate(waits[:-1]):
                        out.append(
                            mybir.InstEventSemaphore(
                                name=f"{inst.name}-hw{j}",
                                engine=inst.engine,
                                ins=[],
                                outs=[],
                                sync_info=type(si)(on_wait=[w], on_update=[]),
                            )
                        )
                        n_fixed += 1
                    si.on_wait = [waits[-1]]
                out.append(inst)
            bb.instructions = out
    return n_fixed


def build(debug=False):
    nc = bass.Bass(num_devices=NC)
    rg = [list(range(NC))]

    # ---------------- dram parameters (per-core shards prepared on host)
    tokloc = nc.declare_dram_parameter("tokloc", [S, 1], i32, isOutput=False)
    tokmsk = nc.declare_dram_parameter("tokmsk", [S, 1], f32, isOutput=False)
    wg = nc.declare_dram_parameter("wg", [VP, D], bf16, isOutput=False)
    wembT = nc.declare_dram_parameter("wembT", [D, VP], bf16, isOutput=False)
    wpos = nc.declare_dram_parameter("wpos", [S, D], bf16, isOutput=False)
    wqT = nc.declare_dram_parameter("wqT", [D, DH], bf16, isOutput=False)
    wkT = nc.declare_dram_parameter("wkT", [D, DH], bf16, isOutput=False)
    wvT = nc.declare_dram_parameter("wvT", [D, DH], bf16, isOutput=False)
    bqs_p = nc.declare_dram_parameter("bqs", [P, 2], f32, isOutput=False)
    bk_p = nc.declare_dram_parameter("bk", [P, 2], f32, isOutput=False)
    bv_p = nc.declare_dram_parameter("bv", [1, DH], f32, isOutput=False)
    w1T = nc.declare_dram_parameter("w1T", [D, DFS], bf16, isOutput=False)
    b1_p = nc.declare_dram_parameter("b1", [P, FT], f32, isOutput=False)
    w2T = nc.declare_dram_parameter("w2T", [DFS, D], bf16, isOutput=False)
    b2_p = nc.declare_dram_parameter("b2", [P, KO], f32, isOutput=False)
    mask_p = nc.declare_dram_parameter("maskbig", [P, 896], bf16, isOutput=False)
    out_p = nc.declare_dram_parameter("out", [S, VP], f32, isOutput=True)
    if debug:
        dbg_xg = nc.declare_dram_parameter("dbg_xg", [S, D], bf16, isOutput=True)
        dbg_at = nc.declare_dram_parameter("dbg_at", [S, DH], bf16, isOutput=True)
        dbg_ot = nc.declare_dram_parameter("dbg_ot", [D, SCW], f32, isOutput=True)
        dbg_q = nc.declare_dram_parameter("dbg_q", [2 * P, S], bf16, isOutput=True)
        dbg_v = nc.declare_dram_parameter("dbg_v", [S, DH + 1], bf16, isOutput=True)

    # ---------------- internal dram (collective bounce buffers)
    xg_in = [nc.dram_tensor(f"xg_in{j}", [SCW, D], bf16) for j in range(SC)]
    xg_out = [
        nc.dram_tensor(f"xg_out{j}", [SCW, D], bf16, addr_space="Shared")
        for j in range(SC)
    ]
    at_in = nc.dram_tensor("at_in", [S, DH], bf16)
    at_out = nc.dram_tensor("at_out", [NC * S, DH], bf16, addr_space="Shared")
    ot_in = [nc.dram_tensor(f"ot_in{j}", [D, SCW], f32) for j in range(SC)]
    ot_out = [
        nc.dram_tensor(f"ot_out{j}", [D, SCW], f32, addr_space="Shared")
        for j in range(SC)
    ]

    with tile.TileContext(nc) as tc:
        with (
            tc.tile_pool(name="glob", bufs=1) as glob,
            tc.tile_pool(name="ps", bufs=6, space="PSUM") as psp,
            tc.tile_pool(name="pss", bufs=2, space="PSUM") as pss,
        ):
            # ---------------- small resident tiles
            msk_sb = glob.tile([P, ST, 1], f32)
            nc.gpsimd.dma_start(
                msk_sb[:], tokmsk.ap().rearrange("(t p) o -> p t o", p=P)
            )
            bqs_sb = glob.tile([P, 2], f32)
            nc.gpsimd.dma_start(bqs_sb[:], bqs_p.ap())
            bk_sb = glob.tile([P, 2], f32)
            nc.gpsimd.dma_start(bk_sb[:], bk_p.ap())
            bv_sb = glob.tile([1, DH], bf16)
            nc.gpsimd.dma_start(bv_sb[:], bv_p.ap())
            ones_sb = glob.tile([1, P], bf16)
            nc.vector.memset(ones_sb[:], 1.0)
            eps_sb = glob.tile([P, 1], f32)
            nc.vector.memset(eps_sb[:], LN_EPS)
            zero_sb = glob.tile([P, 1], f32)
            nc.vector.memset(zero_sb[:], 0.0)
            b1_sb = glob.tile([P, FT], f32)
            nc.gpsimd.dma_start(b1_sb[:], b1_p.ap())
            b2_sb = glob.tile([P, KO], f32)
            nc.gpsimd.dma_start(b2_sb[:], b2_p.ap())
            mask_sb = glob.tile([P, 896], bf16)
            nc.gpsimd.dma_start(mask_sb[:], mask_p.ap())
            wq_sb = glob.tile([P, KO, DH], bf16)
            nc.gpsimd.dma_start(wq_sb[:], wqT.ap().rearrange("(k p) m -> p k m", p=P))
            wk_sb = glob.tile([P, KO, DH], bf16)
            nc.gpsimd.dma_start(wk_sb[:], wkT.ap().rearrange("(k p) m -> p k m", p=P))
            wv_sb = glob.tile([P, KO, DH], bf16)
            nc.gpsimd.dma_start(wv_sb[:], wvT.ap().rearrange("(k p) m -> p k m", p=P))

            # qT/kT [dh, S] and v [S, dh(+1)] persist through attention
            with ExitStack() as attn_stack:
                qkv = attn_stack.enter_context(tc.tile_pool(name="qkv", bufs=1))
                qT_sb = qkv.tile([P, 2, S], bf16)
                kT_sb = qkv.tile([P, 2, S], bf16)
                v_sb = qkv.tile([P, ST, DH + 1], bf16)
                nc.vector.memset(v_sb[:], 1.0)  # col 256 stays 1.0

                with ExitStack() as ph1:
                    emb = ph1.enter_context(tc.tile_pool(name="emb", bufs=3))
                    xt = ph1.enter_context(tc.tile_pool(name="xt", bufs=1))

                    # ---------------- phase 0: masked local embedding gather
                    for i in range(ST):
                        idx_t = emb.tile([P, 1], i32, tag="idx")
                        nc.gpsimd.dma_start(idx_t[:], tokloc.ap()[ts(i, P), :])
                        gat = emb.tile([P, D], bf16, tag="gat")
                        nc.gpsimd.indirect_dma_start(
                            out=gat[:],
                            out_offset=None,
                            in_=wg.ap(),
                            in_offset=bass.IndirectOffsetOnAxis(
                                ap=idx_t[:, :1], axis=0
                            ),
                        )
                        xgp = emb.tile([P, D], bf16, tag="xgp")
                        nc.vector.tensor_scalar_mul(xgp[:], gat[:], msk_sb[:, i, :])
                        wp_t = emb.tile([P, D], bf16, tag="wpp")
                        nc.gpsimd.dma_start(wp_t[:], wpos.ap()[ts(i, P), :])
                        xgp2 = emb.tile([P, D], bf16, tag="xgp2")
                        nc.vector.tensor_add(xgp2[:], xgp[:], wp_t[:])
                        nc.gpsimd.dma_start(
                            xg_in[i // 4].ap()[ts(i % 4, P), :], xgp2[:]
                        )

                    # AllReduce xg (bf16, exact: one-hot masked partials)
                    for j in range(SC):
                        nc.gpsimd.collective_compute(
                            "AllReduce",
                            OP.add,
                            replica_groups=rg,
                            ins=[xg_in[j].ap()],
                            outs=[xg_out[j].ap()],
                        )

                    # ---------------- phase 1: xT load (transposed) + QKV
                    xT_sb = xt.tile([P, KO, S], bf16)
                    for j in range(SC):
                        nc.sync.dma_start_transpose(
                            xT_sb[:, :, ts(j, SCW)],
                            xg_out[j].ap().rearrange("s (k p) -> s k p", p=P),
                        )

                    # qT/kT: [dh(2 Mtiles), s(4 chunks)]
                    for mi in range(2):
                        for j in range(SC):
                            pq = psp.tile([P, SCW], f32, tag="ps")
                            for k in range(KO):
                                nc.tensor.matmul(
                                    pq[:],
                                    wq_sb[:, k, ts(mi, P)],
                                    xT_sb[:, k, ts(j, SCW)],
                                    start=(k == 0),
                                    stop=(k == KO - 1),
                                )
                            nc.scalar.activation(
                                qT_sb[:, mi, ts(j, SCW)], pq[:],
                                AF.Identity,
                                bias=bqs_sb[:, mi : mi + 1],
                                scale=0.0625,
                            )
                            pk = psp.tile([P, SCW], f32, tag="ps")
                            for k in range(KO):
                                nc.tensor.matmul(
                                    pk[:],
                                    wk_sb[:, k, ts(mi, P)],
                                    xT_sb[:, k, ts(j, SCW)],
                                    start=(k == 0),
                                    stop=(k == KO - 1),
                                )
                            nc.scalar.activation(
                                kT_sb[:, mi, ts(j, SCW)], pk[:],
                                AF.Identity,
                                bias=bk_sb[:, mi : mi + 1],
                                scale=1.0,
                            )

                    # v: [s(16 Mtiles), dh] + bv via K=1 ones-matmul
                    for i in range(ST):
                        pv = psp.tile([P, DH], f32, tag="ps")
                        nc.tensor.matmul(
                            pv[:], ones_sb[:], bv_sb[:],
                            start=True, stop=False,
                        )
                        for k in range(KO):
                            nc.tensor.matmul(
                                pv[:],
                                xT_sb[:, k, ts(i, P)],
                                wv_sb[:, k, :],
                                start=False,
                                stop=(k == KO - 1),
                            )
                        nc.vector.tensor_copy(v_sb[:, i, :DH], pv[:])

                # ---------------- phase 2: attention (scoresT -> exp ->
                # mask -> probsT -> attn via [v|1])
                with ExitStack() as ph2:
                    att = ph2.enter_context(tc.tile_pool(name="att", bufs=2))
                    atw = ph2.enter_context(tc.tile_pool(name="atw", bufs=4))
                    for j in range(SC):
                        probs = att.tile([P, ST, SCW], bf16, tag="probs")
                        for m in range(4 * j + 4):
                            psc = psp.tile([P, SCW], f32, tag="ps")
                            for k in range(2):
                                nc.tensor.matmul(
                                    psc[:],
                                    kT_sb[:, k, ts(m, P)],
                                    qT_sb[:, k, ts(j, SCW)],
                                    start=(k == 0),
                                    stop=(k == 1),
                                )
                            nc.scalar.activation(
                                probs[:, m, :], psc[:], AF.Exp,
                                bias=zero_sb[:],
                            )
                            if m >= 4 * j:  # diagonal 128x512 block: causal mask
                                a = 384 - P * (m - 4 * j)
                                nc.vector.tensor_mul(
                                    probs[:, m, :],
                                    probs[:, m, :],
                                    mask_sb[:, ds(a, SCW)],
                                )
                        for il in range(4):
                            i = 4 * j + il
                            pa = pss.tile([P, DH + 1], f32, tag="pa")
                            for m in range(i + 1):
                                nc.tensor.matmul(
                                    pa[:],
                                    probs[:, m, ts(il, P)],
                                    v_sb[:, m, :],
                                    start=(m == 0),
                                    stop=(m == i),
                                )
                            rec = atw.tile([P, 1], f32, tag="rec")
                            nc.vector.reciprocal(rec[:], pa[:, DH : DH + 1])
                            at_t = atw.tile([P, DH], bf16, tag="att")
                            nc.vector.tensor_scalar_mul(at_t[:], pa[:, :DH], rec[:])
                            nc.gpsimd.dma_start(at_in.ap()[ts(i, P), :], at_t[:])

                    if debug:
                        for mi in range(2):
                            nc.gpsimd.dma_start(
                                dbg_q.ap()[ts(mi, P), :], qT_sb[:, mi, :]
                            )
                        for i in range(ST):
                            nc.gpsimd.dma_start(
                                dbg_v.ap()[ts(i, P), :], v_sb[:, i, :]
                            )
                    nc.gpsimd.collective_compute(
                        "AllGather",
                        OP.bypass,
                        replica_groups=rg,
                        ins=[at_in.ap()],
                        outs=[at_out.ap()],
                    )

            # ---------------- phase 3: residual + LN + MLP (per s-chunk)
            with ExitStack() as ph3:
                mlpw = ph3.enter_context(tc.tile_pool(name="mlpw", bufs=1))
                lnw = ph3.enter_context(tc.tile_pool(name="lnw", bufs=2))
                lns = ph3.enter_context(tc.tile_pool(name="lns", bufs=3))
                mlp1 = ph3.enter_context(tc.tile_pool(name="mlp1", bufs=1))
                mlpt = ph3.enter_context(tc.tile_pool(name="mlpt", bufs=2))

                w1_sb = mlpw.tile([P, KO, DFS], bf16)
                nc.gpsimd.dma_start(
                    w1_sb[:], w1T.ap().rearrange("(k p) m -> p k m", p=P)
                )
                w2_sb = mlpw.tile([P, KF, D], bf16)
                nc.gpsimd.dma_start(
                    w2_sb[:], w2T.ap().rearrange("(k p) m -> p k m", p=P)
                )

                for j in range(SC):
                    zT = mlp1.tile([P, KO, SCW], bf16, tag="zT")
                    for il in range(4):
                        i = 4 * j + il
                        xg_t = lnw.tile([P, D], bf16, tag="xg")
                        nc.gpsimd.dma_start(
                            xg_t[:], xg_out[j].ap()[ts(il, P), :]
                        )
                        at_t = lnw.tile([P, NC, DH], bf16, tag="atg")
                        nc.gpsimd.dma_start(
                            at_t[:],
                            at_out.ap().rearrange("(c s) d -> s c d", s=S)[
                                ts(i, P), :, :
                            ],
                        )
                        t1 = lns.tile([P, D], f32, tag="hbuf")
                        h_t = lns.tile([P, D], f32, tag="hbuf")
                        nc.vector.tensor_add(
                            h_t[:], xg_t[:], at_t[:].rearrange("p c d -> p (c d)")
                        )
                        hsum = lns.tile([P, 1], f32, tag="hsum")
                        nc.vector.tensor_reduce(
                            out=hsum[:], in_=h_t[:],
                            axis=mybir.AxisListType.X, op=OP.add,
                        )
                        ss = lns.tile([P, 1], f32, tag="ss")
                        nc.scalar.activation(
                            t1[:], h_t[:], AF.Square,
                            bias=zero_sb[:], accum_out=ss[:],
                        )
                        negmu = lns.tile([P, 1], f32, tag="negmu")
                        nc.scalar.mul(negmu[:], hsum[:], -1.0 / D)
                        mu2 = lns.tile([P, 1], f32, tag="mu2")
                        nc.vector.tensor_mul(mu2[:], negmu[:], negmu[:])
                        var0 = lns.tile([P, 1], f32, tag="var0")
                        nc.vector.tensor_scalar_mul(var0[:], ss[:], 1.0 / D)
                        varm = lns.tile([P, 1], f32, tag="varm")
                        nc.vector.tensor_sub(varm[:], var0[:], mu2[:])
                        stdt = lns.tile([P, 1], f32, tag="stdt")
                        nc.scalar.activation(
                            stdt[:], varm[:], AF.Sqrt, bias=eps_sb[:], scale=1.0
                        )
                        rstd = lns.tile([P, 1], f32, tag="rstd")
                        nc.vector.reciprocal(rstd[:], stdt[:])
                        z_bf = lns.tile([P, D], bf16, tag="z")
                        nc.vector.tensor_scalar(
                            out=z_bf[:], in0=h_t[:],
                            scalar1=negmu[:], scalar2=rstd[:],
                            op0=OP.add, op1=OP.mult,
                        )
                        nc.sync.dma_start_transpose(
                            zT[:, :, ts(il, P)],
                            z_bf[:].rearrange("s (k p) -> s k p", p=P),
                        )

                    # MLP1: mT[f, s-chunk] = gelu(W1' @ zT + b1')
                    mT = mlp1.tile([P, KF, SCW], bf16, tag="mT")
                    for fi in range(FT):
                        pm = psp.tile([P, SCW], f32, tag="ps")
                        for k in range(KO):
                            nc.tensor.matmul(
                                pm[:],
                                w1_sb[:, k, ts(fi, P)],
                                zT[:, k, :],
                                start=(k == 0),
                                stop=(k == KO - 1),
                            )
                        nc.scalar.activation(
                            mT[:, fi, :], pm[:], AF.Gelu,
                            bias=b1_sb[:, fi : fi + 1], scale=1.0,
                        )
                    # MLP2: oT[d, s-chunk] partial = W2'T @ mT
                    for di in range(KO):
                        po = psp.tile([P, SCW], f32, tag="ps")
                        for k in range(KF):
                            nc.tensor.matmul(
                                po[:],
                                w2_sb[:, k, ts(di, P)],
                                mT[:, k, :],
                                start=(k == 0),
                                stop=(k == KF - 1),
                            )
                        ot_t = mlpt.tile([P, SCW], f32, tag="ot")
                        nc.vector.tensor_copy(ot_t[:], po[:])
                        nc.gpsimd.dma_start(ot_in[j].ap()[ts(di, P), :], ot_t[:])

                    nc.gpsimd.collective_compute(
                        "AllReduce",
                        OP.add,
                        replica_groups=rg,
                        ins=[ot_in[j].ap()],
                        outs=[ot_out[j].ap()],
                    )

            if debug:
                with ExitStack() as phd:
                    dbgp = phd.enter_context(tc.tile_pool(name="dbgp", bufs=2))
                    for j in range(SC):
                        dt_ = dbgp.tile([P, 4, D], bf16, tag="dxg")
                        nc.gpsimd.dma_start(dt_[:], xg_out[j].ap().rearrange("(a p) d -> p a d", p=P))
                        nc.gpsimd.dma_start(dbg_xg.ap().rearrange("(c a p) d -> p a d", p=P, a=4)[:, :, :].rearrange("p a d -> p a d") if False else dbg_xg.ap()[ds(j * SCW, SCW), :].rearrange("(a p) d -> p a d", p=P), dt_[:])
                    for i in range(ST):
                        da = dbgp.tile([P, DH], bf16, tag="dat")
                        nc.gpsimd.dma_start(da[:], at_in.ap()[ts(i, P), :])
                        nc.gpsimd.dma_start(dbg_at.ap()[ts(i, P), :], da[:])
                    for di in range(KO):
                        do_ = dbgp.tile([P, SCW], f32, tag="dot")
                        nc.gpsimd.dma_start(do_[:], ot_out[0].ap()[ts(di, P), :])
                        nc.gpsimd.dma_start(dbg_ot.ap()[ts(di, P), :], do_[:])

            # ---------------- phase 4: lm_head (vocab-sharded)
            with ExitStack() as ph4:
                lmw = ph4.enter_context(tc.tile_pool(name="lmw", bufs=1))
                lms = ph4.enter_context(tc.tile_pool(name="lms", bufs=3))
                lmo = ph4.enter_context(tc.tile_pool(name="lmo", bufs=4))

                oT_all = lmw.tile([P, KO, S], bf16)
                for j in range(SC):
                    nc.gpsimd.dma_start(
                        oT_all[:, :, ts(j, SCW)],
                        ot_out[j].ap().rearrange("(k p) s -> p k s", p=P),
                    )
                    nc.vector.tensor_tensor(
                        out=oT_all[:, :, ts(j, SCW)],
                        in0=oT_all[:, :, ts(j, SCW)],
                        in1=b2_sb[:, :, None].to_broadcast([P, KO, SCW]),
                        op=OP.add,
                    )

                vchunks = [(a, min(512, VP - a)) for a in range(0, VP, 512)]
                for g in range(2):
                    for (va, vn) in vchunks:
                        wt = lms.tile([P, KO, 512], bf16, tag="wt")
                        nc.gpsimd.dma_start(
                            wt[:, :, :vn],
                            wembT.ap().rearrange("(k p) v -> p k v", p=P)[
                                :, :, ds(va, vn)
                            ],
                        )
                        for si in range(8 * g, 8 * g + 8):
                            pl = psp.tile([P, SCW], f32, tag="ps")
                            for k in range(KO):
                                nc.tensor.matmul(
                                    pl[:, :vn],
                                    oT_all[:, k, ts(si, P)],
                                    wt[:, k, :vn],
                                    start=(k == 0),
                                    stop=(k == KO - 1),
                                )
                            lo = lmo.tile([P, 512], f32, tag="lo")
                            nc.vector.tensor_copy(lo[:, :vn], pl[:, :vn])
                            nc.gpsimd.dma_start(
                                out_p.ap()[ts(si, P), ds(va, vn)], lo[:, :vn]
                            )

    fix_wait_limits(nc)
    return nc


_CACHE = {}


def _prep_inputs(token_ids, Wemb, Wpos, Wq, bq, Wk, bk, Wv, bv,
                 ln_g, ln_b, W1, b1, W2, b2):
    """Host-side sharding / layout prep. Returns in_maps for 8 cores."""
    tok = np.asarray(token_ids).astype(np.int64)
    Wemb = np.asarray(Wemb, dtype=np.float32)
    wemb_pad = np.zeros((VPAD, D), np.float32)
    wemb_pad[:V] = Wemb

    # LN affine folded into W1/b1 (no residual after the MLP)
    W1 = np.asarray(W1, dtype=np.float32)
    ln_g = np.asarray(ln_g, dtype=np.float32)
    ln_b = np.asarray(ln_b, dtype=np.float32)
    W1f = W1 * ln_g[None, :]
    b1f = np.asarray(b1, dtype=np.float32) + W1 @ ln_b

    W2 = np.asarray(W2, dtype=np.float32)
    Wpos = np.asarray(Wpos, dtype=np.float32)

    # causal staircase mask for the diagonal 128x512 blocks:
    # maskbig[t, u] = 1 iff u >= t + 384; slice [a:a+512], a = 384-128*r
    u = np.arange(896)[None, :]
    t = np.arange(P)[:, None]
    maskbig = (u >= t + 384).astype(ml_dtypes.bfloat16)

    in_maps = []
    for c in range(NC):
        vlo = c * VP
        wsh = wemb_pad[vlo : vlo + VP]
        hs = slice(c * DH, (c + 1) * DH)
        fs = slice(c * DFS, (c + 1) * DFS)
        tl = np.clip(tok - vlo, 0, VP - 1).astype(np.int32)[:, None]
        tm = ((tok >= vlo) & (tok < vlo + VP)).astype(np.float32)[:, None]
        in_maps.append({
            "tokloc": tl,
            "tokmsk": tm,
            "wg": wsh.astype(ml_dtypes.bfloat16),
            "wembT": np.ascontiguousarray(wsh.T).astype(ml_dtypes.bfloat16),
            "wpos": (Wpos if c == 0 else np.zeros_like(Wpos)).astype(ml_dtypes.bfloat16),
            "wqT": np.ascontiguousarray(np.asarray(Wq, np.float32)[hs].T).astype(ml_dtypes.bfloat16),
            "wkT": np.ascontiguousarray(np.asarray(Wk, np.float32)[hs].T).astype(ml_dtypes.bfloat16),
            "wvT": np.ascontiguousarray(np.asarray(Wv, np.float32)[hs].T).astype(ml_dtypes.bfloat16),
            "bqs": np.ascontiguousarray(
                (np.asarray(bq, np.float32)[hs] / 16.0).reshape(2, P).T
            ),
            "bk": np.ascontiguousarray(
                np.asarray(bk, np.float32)[hs].reshape(2, P).T
            ),
            "bv": np.asarray(bv, np.float32)[hs][None, :],
            "w1T": np.ascontiguousarray(W1f[fs].T).astype(ml_dtypes.bfloat16),
            "b1": np.ascontiguousarray(b1f[fs].reshape(FT, P).T),
            "w2T": np.ascontiguousarray(W2[:, fs].T).astype(ml_dtypes.bfloat16),
            "b2": np.ascontiguousarray(
                np.asarray(b2, np.float32).reshape(KO, P).T
            ),
            "maskbig": maskbig,
        })
    return in_maps


def _get_nc():
    if "nc" not in _CACHE:
        _CACHE["nc"] = build()
    return _CACHE["nc"]


def kernel(**inputs) -> np.ndarray:
    nc = _get_nc()
    in_maps = _prep_inputs(**inputs)
    res = run_bass_kernel_spmd(nc, in_maps, core_ids=list(range(NC)))
    out = np.concatenate([r["out"] for r in res.results], axis=1)
    return out[:, :V]
